# revision 6
# baseline (speedup 1.0000x reference)
"""AFNB (asymmetric fusion non-local block) Trainium2 kernel, 8 NeuronCores.

Sharding: core c handles batch b = c//2, spatial row-half s = c%2 (36 of 72 rows).
The PSP-pooled key/value need the full spatial extent of the batch element:
each core convs only its own half, pools it to a 3x3-px cell grid, and a
pair-wise AllGather ([0,1],[2,3],...) exchanges the half-grids; both cores then
finish the pooling from the full grid. Queries / fused output are local.

Algebra done on the host (exact, fp64):
  - BatchNorm folded into conv weights/biases everywhere.
  - Fusion conv split over the channel-concat: wf @ [ctx; high] = wf_c@ctx + wf_h@high.
  - wf_c @ (ww @ attn_out + bw) folded to Wcc = (wf_c*sf) @ ww  (2048x256) and a bias,
    which removes the explicit 2048-channel "context" tensor entirely.
  - softmax 1/16 scale folded into the query bias/scale; PSP mean 1/binsize folded
    into pooled key/value scaling.

Matmuls run in bf16 (inputs pre-cast host-side) with fp32 PSUM accumulation.
Phase B is software-pipelined: chunk c+1's attention stages are interleaved
between chunk c's fused-conv ct-groups so the PE never waits on the
exp/reciprocal/broadcast serial chain; queries run 3 chunks ahead so the PE
has work while the pooling AllGather is in flight.
"""

import numpy as np
import ml_dtypes

B, CL, CH, CK, CV, CO = 4, 1024, 2048, 256, 256, 2048
H = W = 72
N = H * W          # 5184
HL = 36            # rows per core
NL = HL * W        # 2592
PSP = (1, 3, 6, 8)
M = 110            # pooled locations
EPS = 1e-5
NCORES = 8
KL, KH = CL // 128, CH // 128   # 8, 16 contraction tiles
NT = 432                         # moving-dim tile (6 rows of 72)
NTILES_A = NL // NT              # 6 (local half only)
NCH_B = NL // NT                 # 6
GC = 288                         # local grid cells: 12 rows x 24 cols (3x3 px)

_CACHE = {}
LAST_RESULTS = None


def _build_graph():
    import concourse.bacc as bacc
    import concourse.mybir as mybir
    from concourse import tile, masks

    F32 = mybir.dt.float32
    BF16 = mybir.dt.bfloat16
    AF = mybir.ActivationFunctionType
    AX = mybir.AxisListType

    nc = bacc.Bacc("TRN2", target_bir_lowering=False, debug=False, num_devices=NCORES)

    low_d = nc.dram_tensor("low", [128, KL, NL], BF16, kind="ExternalInput").ap()
    high_d = nc.dram_tensor("high", [128, KH, NL], BF16, kind="ExternalInput").ap()
    wlow_d = nc.dram_tensor("wlow", [128, KL, 512], BF16, kind="ExternalInput").ap()
    whigh_d = nc.dram_tensor("whigh", [128, KH, 2304], BF16, kind="ExternalInput").ap()
    wcc_d = nc.dram_tensor("wcc", [128, 2, CO], BF16, kind="ExternalInput").ap()
    blow_d = nc.dram_tensor("blow", [128, 4], F32, kind="ExternalInput").ap()
    bq_d = nc.dram_tensor("bq", [128, 2], F32, kind="ExternalInput").ap()
    bout_d = nc.dram_tensor("bout", [128, 16], F32, kind="ExternalInput").ap()
    binv_d = nc.dram_tensor("binv", [128, M], F32, kind="ExternalInput").ap()
    out_d = nc.dram_tensor("out", [16, 128, NL], F32, kind="ExternalOutput").ap()

    with tile.TileContext(nc) as tc:
        with (
            tc.tile_pool(name="wp", bufs=1) as wp,
            tc.tile_pool(name="cp", bufs=1) as cp,
            tc.tile_pool(name="lp", bufs=2) as lp,
            tc.tile_pool(name="cvp", bufs=3) as cvp,
            tc.tile_pool(name="gp", bufs=1) as gp,
            tc.tile_pool(name="pp", bufs=1) as pp,
            tc.tile_pool(name="hp", bufs=4) as hp,
            tc.tile_pool(name="ab", bufs=2) as ab,
            tc.tile_pool(name="qp", bufs=3) as qp,
            tc.tile_pool(name="op", bufs=4) as op_,
            tc.tile_pool(name="dp", bufs=1, space="DRAM") as dp,
            tc.tile_pool(name="ps", bufs=4, space="PSUM") as ps,
            tc.tile_pool(name="psO", bufs=4, space="PSUM") as psO,
        ):
            # ---- phase-A weights & constants first (small, unblock PE fast) ----
            wlow_sb = wp.tile([128, KL * 512], BF16, name="wlow_sb").rearrange(
                "p (k m) -> p k m", k=KL)
            for k in range(KL):
                nc.sync.dma_start(out=wlow_sb[:, k], in_=wlow_d[:, k])
            blow_sb = cp.tile([128, 4], F32, name="blow_sb")
            nc.sync.dma_start(out=blow_sb, in_=blow_d)
            bq_sb = cp.tile([128, 2], F32, name="bq_sb")
            nc.sync.dma_start(out=bq_sb, in_=bq_d)
            bout_sb = cp.tile([128, 16], F32, name="bout_sb")
            nc.sync.dma_start(out=bout_sb, in_=bout_d)
            binv_sb = cp.tile([128, M], F32, name="binv_sb")
            nc.sync.dma_start(out=binv_sb, in_=binv_d)
            ident = cp.tile([128, 128], F32, name="ident")
            masks.make_identity(nc, ident)
            ones110 = cp.tile([110, 1], BF16, name="ones110")
            nc.vector.memset(ones110, 1.0)
            ones1 = cp.tile([1, 128], BF16, name="ones1")
            nc.vector.memset(ones1, 1.0)

            # ---- phase A: low conv (k|v, 512 ch) over local half + grid pool ----
            grid = [gp.tile([128, GC], F32, name=f"grid{t}") for t in range(4)]
            for nt in range(NTILES_A):
                low_c = lp.tile([128, KL * NT], BF16, name="low_c",
                                tag="low_c").rearrange("p (k n) -> p k n", k=KL)
                for k in range(KL):
                    nc.sync.dma_start(out=low_c[:, k],
                                      in_=low_d[:, k, nt * NT:(nt + 1) * NT])
                for ct in range(4):
                    cps = ps.tile([128, NT], F32, name="cv_ps", tag="ps")
                    for k in range(KL):
                        nc.tensor.matmul(cps, wlow_sb[:, k, ct * 128:(ct + 1) * 128],
                                         low_c[:, k, :],
                                         start=(k == 0), stop=(k == KL - 1))
                    cv = cvp.tile([128, NT], F32, name="cv", tag="cv")
                    nc.scalar.activation(cv, cps,
                                         AF.Relu if ct < 2 else AF.Identity,
                                         bias=blow_sb[:, ct:ct + 1])
                    # 6 rows x 72 cols -> 2 grid-rows x 24 grid-cols (3x3 cells)
                    cvv = cv.rearrange("p (gi ri gj cj) -> p gi gj ri cj",
                                       gi=2, ri=3, gj=24, cj=3)
                    gsl = grid[ct][:, nt * 48:(nt + 1) * 48].rearrange(
                        "p (gi gj) -> p gi gj", gi=2)
                    for gi in range(2):
                        nc.vector.reduce_sum(gsl[:, gi], cvv[:, gi], axis=AX.XY)

            # exchange half-grids with the pair core: AllGather over [2b, 2b+1]
            cc_in = dp.tile([4, 128, GC], F32, name="cc_in")
            cc_out = dp.tile([2, 4, 128, GC], F32, name="cc_out")
            for t in range(4):
                nc.sync.dma_start(out=cc_in[t], in_=grid[t])
            nc.gpsimd.collective_compute(
                "AllGather", mybir.AluOpType.bypass,
                replica_groups=[[0, 1], [2, 3], [4, 5], [6, 7]],
                ins=[cc_in.opt()], outs=[cc_out.opt()])
            # gfull[t]: [128, 2 (rank=row half), 12 grid rows, 24 grid cols]
            gfull = [gp.tile([128, 2 * GC], F32, name=f"gfull{t}").rearrange(
                "p (r gi gj) -> p r gi gj", r=2, gi=12) for t in range(4)]
            for t in range(4):
                for r in range(2):
                    nc.sync.dma_start(out=gfull[t][:, r], in_=cc_out[r, t])

            # stage-2: full 24x24 grid -> 110 pooled sums per channel tile
            pooled = [pp.tile([128, M], F32, name=f"pooled{t}") for t in range(4)]
            tmp = pp.tile([128, 16], F32, name="tmp_pool")

            def red_rows(t, r, a, b, s, dst):
                # sum rows [a,b) of rank r's half-grid into s column bins -> dst
                v = gfull[t][:, r, a:b, :].rearrange(
                    "p ri (bj cj) -> p bj ri cj", bj=s)
                nc.vector.reduce_sum(dst, v, axis=AX.XY)

            for t in range(4):
                # s=1 (bin rows span both halves)
                red_rows(t, 0, 0, 12, 1, pooled[t][:, 0:1])
                red_rows(t, 1, 0, 12, 1, tmp[:, 0:1])
                nc.vector.tensor_add(pooled[t][:, 0:1], pooled[t][:, 0:1],
                                     tmp[:, 0:1])
                # s=3: bin rows of 8 grid-rows; middle bin straddles
                red_rows(t, 0, 0, 8, 3, pooled[t][:, 1:4])
                red_rows(t, 0, 8, 12, 3, pooled[t][:, 4:7])
                red_rows(t, 1, 0, 4, 3, tmp[:, 0:3])
                nc.vector.tensor_add(pooled[t][:, 4:7], pooled[t][:, 4:7],
                                     tmp[:, 0:3])
                red_rows(t, 1, 4, 12, 3, pooled[t][:, 7:10])
                # s=6: bin rows of 4 grid-rows; no straddle
                for bi in range(3):
                    red_rows(t, 0, bi * 4, bi * 4 + 4, 6,
                             pooled[t][:, 10 + bi * 6: 16 + bi * 6])
                    red_rows(t, 1, bi * 4, bi * 4 + 4, 6,
                             pooled[t][:, 28 + bi * 6: 34 + bi * 6])
                # s=8: bin rows of 3 grid-rows; no straddle
                for bi in range(4):
                    red_rows(t, 0, bi * 3, bi * 3 + 3, 8,
                             pooled[t][:, 46 + bi * 8: 54 + bi * 8])
                    red_rows(t, 1, bi * 3, bi * 3 + 3, 8,
                             pooled[t][:, 78 + bi * 8: 86 + bi * 8])

            # scale by 1/binsize; key -> bf16 directly, value -> f32 then transpose
            k_bf = [pp.tile([128, M], BF16, name=f"kbf{t}") for t in range(2)]
            for t in range(2):
                nc.vector.tensor_mul(k_bf[t], pooled[t], binv_sb)
            v_sc = [pp.tile([128, M], F32, name=f"vsc{t}") for t in range(2)]
            for t in range(2):
                nc.vector.tensor_mul(v_sc[t], pooled[2 + t], binv_sb)
            VT = [pp.tile([110, 128], BF16, name=f"VT{t}") for t in range(2)]
            for t in range(2):
                tp = ps.tile([110, 128], F32, name="vt_ps", tag="ps")
                nc.tensor.transpose(tp, v_sc[t], ident)
                nc.scalar.copy(VT[t], tp)

            # ---- phase-B weights (big; after phase A so they don't block it) ----
            whigh_sb = wp.tile([128, KH * 2304], BF16, name="whigh_sb").rearrange(
                "p (k m) -> p k m", k=KH)
            for k in range(KH):
                nc.sync.dma_start(out=whigh_sb[:, k], in_=whigh_d[:, k])
            wcc_sb = wp.tile([128, 2 * CO], BF16, name="wcc_sb").rearrange(
                "p (k m) -> p k m", k=2)
            nc.sync.dma_start(out=wcc_sb, in_=wcc_d)

            # ---- phase B: software-pipelined q/attention + fused output conv ----
            st = {}  # per-chunk live tiles

            def load_high(c):
                hc = hp.tile([128, KH * NT], BF16, name="high_c",
                             tag="high_c").rearrange("p (k n) -> p k n", k=KH)
                nc.sync.dma_start(out=hc, in_=high_d[:, :, c * NT:(c + 1) * NT])
                st[c] = {'high': hc}

            def emit_q(c):
                hc = st[c]['high']
                q_sb = qp.tile([128, 2 * NT], BF16, name="q_sb",
                               tag="q_sb").rearrange("p (t n) -> p t n", t=2)
                for qt in range(2):
                    qps = ps.tile([128, NT], F32, name="q_ps", tag="ps")
                    for k in range(KH):
                        nc.tensor.matmul(qps,
                                         whigh_sb[:, k, qt * 128:(qt + 1) * 128],
                                         hc[:, k, :],
                                         start=(k == 0), stop=(k == KH - 1))
                    nc.scalar.activation(q_sb[:, qt], qps, AF.Relu,
                                         bias=bq_sb[:, qt:qt + 1], scale=0.0625)
                st[c]['q'] = q_sb

            def emit_sim(c):
                sim_ps = ps.tile([110, NT], F32, name="sim_ps", tag="ps")
                for t in range(2):
                    nc.tensor.matmul(sim_ps, k_bf[t], st[c]['q'][:, t],
                                     start=(t == 0), stop=(t == 1))
                E = ab.tile([110, NT], BF16, name="E", tag="E")
                nc.scalar.activation(E, sim_ps, AF.Exp)
                st[c]['E'] = E

            def emit_s(c):
                s_ps = ps.tile([1, NT], F32, name="s_ps", tag="ps")
                nc.tensor.matmul(s_ps, ones110, st[c]['E'], start=True, stop=True)
                s_sb = ab.tile([1, NT], F32, name="s_sb", tag="s_sb")
                nc.scalar.copy(s_sb, s_ps)
                r_sb = ab.tile([1, NT], F32, name="r_sb", tag="r_sb")
                nc.vector.reciprocal_approx_fast(r_sb, s_sb)
                r_bf = ab.tile([1, NT], BF16, name="r_bf", tag="r_bf")
                nc.vector.tensor_copy(r_bf, r_sb)
                st[c]['r'] = r_bf

            def emit_rbc(c):
                rbc_ps = ps.tile([128, NT], F32, name="rbc_ps", tag="ps")
                nc.tensor.matmul(rbc_ps, ones1, st[c]['r'], start=True, stop=True)
                rbc_sb = ab.tile([128, NT], F32, name="rbc_sb", tag="rbc_sb")
                nc.scalar.copy(rbc_sb, rbc_ps)
                st[c]['rbc'] = rbc_sb

            def emit_ctx(c):
                ctx_bf = ab.tile([128, 2 * NT], BF16, name="ctx_bf",
                                 tag="ctx_bf").rearrange("p (t n) -> p t n", t=2)
                for t in range(2):
                    cps = ps.tile([128, NT], F32, name="ctx_ps", tag="ps")
                    nc.tensor.matmul(cps, VT[t], st[c]['E'], start=True, stop=True)
                    nc.vector.tensor_mul(ctx_bf[:, t], cps, st[c]['rbc'])
                st[c]['ctx'] = ctx_bf

            def emit_fused(c, cts):
                hc, ctx = st[c]['high'], st[c]['ctx']
                for ct in cts:
                    ops = psO.tile([128, NT], F32, name="o_ps", tag="psO")
                    for k in range(KH):
                        nc.tensor.matmul(
                            ops,
                            whigh_sb[:, k, 256 + ct * 128:256 + (ct + 1) * 128],
                            hc[:, k, :], start=(k == 0), stop=False)
                    for t in range(2):
                        nc.tensor.matmul(ops, wcc_sb[:, t, ct * 128:(ct + 1) * 128],
                                         ctx[:, t], start=False, stop=(t == 1))
                    o_sb = op_.tile([128, NT], F32, name="o_sb", tag="o_sb")
                    nc.scalar.activation(o_sb, ops, AF.Identity,
                                         bias=bout_sb[:, ct:ct + 1])
                    nc.sync.dma_start(out=out_d[ct, :, c * NT:(c + 1) * NT],
                                      in_=o_sb)

            for c in range(NCH_B):
                load_high(c)
            # prologue: 3 chunks of query conv keep the PE busy while the
            # pooling AllGather completes; then attention for chunk 0
            emit_q(0)
            emit_q(1)
            emit_q(2)
            emit_sim(0)
            emit_s(0)
            emit_rbc(0)
            emit_ctx(0)
            for c in range(NCH_B):
                n = c + 1
                if n < NCH_B:
                    if c + 3 < NCH_B:
                        emit_q(c + 3)
                    emit_fused(c, range(0, 4))
                    emit_sim(n)
                    emit_fused(c, range(4, 7))
                    emit_s(n)
                    emit_fused(c, range(7, 11))
                    emit_rbc(n)
                    emit_fused(c, range(11, 13))
                    emit_ctx(n)
                    emit_fused(c, range(13, 16))
                else:
                    emit_fused(c, range(16))
                del st[c]

    nc.compile()
    return nc


def _pack_tiles(a2d, ktiles):
    """(ktiles*128, F) -> [128, ktiles, F] contiguous."""
    k128, F = a2d.shape
    assert k128 == ktiles * 128
    return np.ascontiguousarray(a2d.reshape(ktiles, 128, F).transpose(1, 0, 2))


def _bf16(a):
    return np.ascontiguousarray(a).astype(ml_dtypes.bfloat16)


def _prep_consts(inputs):
    f64 = {k: np.asarray(v, np.float64) for k, v in inputs.items()}
    sk = f64['gk'] / np.sqrt(f64['vk'] + EPS)
    sq = f64['gq'] / np.sqrt(f64['vq'] + EPS)
    sf = f64['gf'] / np.sqrt(f64['vf'] + EPS)

    wk_f = f64['wk'] * sk[:, None]
    bk_f = (f64['bk'] - f64['mk']) * sk + f64['bek']
    wq_f = f64['wq'] * sq[:, None]
    bq_f = ((f64['bq'] - f64['mq']) * sq + f64['beq']) / 16.0

    wf_s = f64['wf'] * sf[:, None]
    A = wf_s[:, :CO]          # context part (2048, 2048)
    Bh = wf_s[:, CO:]         # high part (2048, 2048)
    Wcc = A @ f64['ww']       # (2048, 256)
    bout = A @ f64['bw'] + (f64['bf'] - f64['mf']) * sf + f64['bef']

    wlowT = np.concatenate([wk_f, f64['wv']], axis=0).T      # (1024, 512)
    whighT = np.concatenate([wq_f, Bh], axis=0).T            # (2048, 2304)
    wccT = Wcc.T                                             # (256, 2048)
    blow = np.concatenate([bk_f, f64['bv']])                 # (512,)

    binv = np.concatenate(
        [np.full(s * s, 1.0 / ((H // s) * (W // s))) for s in PSP])  # (110,)

    return {
        'wlow': _bf16(_pack_tiles(wlowT, KL)),
        'whigh': _bf16(_pack_tiles(whighT, KH)),
        'wcc': _bf16(_pack_tiles(wccT, 2)),
        'blow': np.ascontiguousarray(blow.reshape(4, 128).T, dtype=np.float32),
        'bq': np.ascontiguousarray(bq_f.reshape(2, 128).T, dtype=np.float32),
        'bout': np.ascontiguousarray(bout.reshape(16, 128).T, dtype=np.float32),
        'binv': np.ascontiguousarray(
            np.broadcast_to(binv, (128, M)), dtype=np.float32),
    }


def kernel(**inputs):
    global LAST_RESULTS
    from concourse.bass_utils import run_bass_kernel_spmd

    if 'nc' not in _CACHE:
        _CACHE['nc'] = _build_graph()
    nc = _CACHE['nc']

    consts = _prep_consts(inputs)
    low_feats = np.asarray(inputs['low_feats'], np.float32)
    high_feats = np.asarray(inputs['high_feats'], np.float32)

    in_maps = []
    for c in range(NCORES):
        b, s = c // 2, c % 2
        low2d = low_feats[b, :, s * HL:(s + 1) * HL, :].reshape(CL, NL)
        high2d = high_feats[b, :, s * HL:(s + 1) * HL, :].reshape(CH, NL)
        m = dict(consts)
        m['low'] = _bf16(_pack_tiles(low2d, KL))
        m['high'] = _bf16(_pack_tiles(high2d, KH))
        in_maps.append(m)

    res = run_bass_kernel_spmd(nc, in_maps, list(range(NCORES)))
    LAST_RESULTS = res

    out = np.empty((B, CO, H, W), np.float32)
    for c in range(NCORES):
        b, s = c // 2, c % 2
        o = np.asarray(res.results[c]['out'], np.float32).reshape(CO, HL, W)
        out[b, :, s * HL:(s + 1) * HL, :] = o
    return out
